# revision 1
# baseline (speedup 1.0000x reference)
"""AdaptiveCompressionLayer kernel for 8 TRN2 NeuronCores.

Strategy (expert-routed data parallel):
  - Host: bucket tokens by importance score (>0.8 / >0.4 / rest), gather
    tokens into per-expert groups, split evenly across 8 cores with fixed
    per-expert capacities, and pre-transpose the routed activations to
    [H, T_pad] so the device needs no on-chip transposes.
  - Device (SPMD, identical graph on all 8 cores): for each 512-token
    group (single expert per group, known at compile time):
        Z^T = Wc^T @ X^T   (PE, bf16, f32 PSUM accumulation)
        Z^T += bc          (ScalarE activation copy w/ per-partition bias)
        Y   = Z^T.T @ [Wd; bd]  (ones-row trick folds bd into the matmul)
        out = LayerNorm(Y) (bn_stats/bn_aggr + sqrt/recip + affine), bf16
  - Host: scatter valid rows back to the original token order.

PE ragged-tail packing (the hc dims 691/537/76 are not multiples of 128):
  - mm1 tail M-chunks (51/25 cols) of 2-4 different groups run CONCURRENTLY
    as col-tiles (tile_position=(0, 32j)) in one PE pass.
  - mm2 tail K-chunks (52/26 rows) of 2 subtiles run concurrently as
    row-tiles (tile_position=(32i, 0)); the z tail is replicated to a
    second partition offset by an SBUF->SBUF DMA, the wd tail rows are
    host-replicated at every offset.

No cross-core communication: routing is per-token, weights replicated.
"""
import sys

sys.path.insert(0, "/opt/trn_rl_repo")

import numpy as np
import ml_dtypes

BF16 = ml_dtypes.bfloat16

H = 768
HC = (691, 537, 76)
S = 65536
EPS = 1e-5
N_CORES = 8
GROUP = 512
CAPS = (1792, 3328, 3328)  # default; kernel() tightens from actual counts
# chunk counts along hc (mm1 M-chunks == mm2 K-chunks; bias row fits in last)
MC = tuple((hc + 127) // 128 for hc in HC)  # (6, 5, 1)
FM = tuple(hc // 128 for hc in HC)  # full 128-col chunks (5, 4, 0)
TM = tuple(hc % 128 for hc in HC)  # tail cols (51, 25, 76)
TK = tuple(t + 1 for t in TM)  # tail contraction rows incl. ones (52, 26, 77)
SLOTW = (64, 32, 128)  # col-tile width holding the mm1 tail per expert

TRACE = False
LAST_RESULT = None

_NC_CACHE = {}


def _groups(caps):
    per_e = []
    offs = (0, caps[0], caps[0] + caps[1])
    for e in range(3):
        glist = []
        t = 0
        while t < caps[e]:
            gsz = min(GROUP, caps[e] - t)
            glist.append((e, offs[e] + t, gsz))
            t += gsz
        per_e.append(glist)
    return per_e


def _rounds(caps):
    """Rounds of same-expert groups; each round's mm1 tails get packed into
    one col-tiled PE pass. e1 rounds up to 4 (32-col slots), e0 up to 2
    (64-col), e2 singles (76 cols round up to the full array)."""
    per_e = [list(g) for g in _groups(caps)]
    rounds = []
    if per_e[2]:
        rounds.append([per_e[2].pop(0)])
    # e2 groups go early/mid (their LayerNorm debt needs other experts' PE
    # work to hide under); finish on e1 then e0 so the drain is PE-heavy,
    # with the smallest group last.
    cycle = ((1, 4), (2, 1), (2, 1), (0, 2), (2, 1), (2, 1), (1, 4), (2, 1), (2, 1), (0, 2))
    ci = 0
    while any(per_e):
        e, n = cycle[ci % len(cycle)]
        ci += 1
        src = per_e[e]
        if not src:
            continue
        rounds.append([src.pop(0) for _ in range(min(n, len(src)))])
    return rounds



def _order(caps):
    """Flat group order: e2 first (tiny weights), then PE-heavy e1/e0
    groups with DVE-heavy e2 singles spread between them (an e2 group has
    ~2.6x more LayerNorm work than its own matmuls can hide); ends on the
    smallest group so the final LayerNorm+store tail is short."""
    per_e = _groups(caps)
    e0g, e1g, e2g = per_e[0], per_e[1], per_e[2]
    # heavies: interleave e0 among e1 (both orders keep big groups first),
    # smallest-last overall
    heavy = []
    h1, h0 = list(e1g), list(e0g)
    # keep the two smallest (any sub-512) for the very end
    tail_small = [g for g in h0 if g[2] < 512][-1:]
    h0 = [g for g in h0 if g not in tail_small]
    ratio = max(1, len(h1) // max(1, len(h0)))
    while h1 or h0:
        for _ in range(ratio):
            if h1:
                heavy.append(h1.pop(0))
        if h0:
            heavy.append(h0.pop(0))
    heavy.extend(tail_small)
    light = list(e2g)
    order = []
    if light:
        order.append(light.pop(0))
    hi = 0
    # two heavies up front (weights for e1 arrive during e2+e1 work),
    # then alternate heavy/light until lights run out
    nburst = 2
    while hi < len(heavy) or light:
        for _ in range(nburst):
            if hi < len(heavy):
                order.append(heavy[hi])
                hi += 1
        nburst = 1
        if light:
            order.append(light.pop(0))
    return order


def _first_use_order(caps):
    seen = []
    for e, _, _ in _order(caps):
        if e not in seen:
            seen.append(e)
    return seen


def _build(apply_gb: bool, caps=CAPS):
    import concourse.mybir as mybir
    import concourse.tile as tile
    from concourse import bacc

    f32 = mybir.dt.float32
    bf16 = mybir.dt.bfloat16
    AF = mybir.ActivationFunctionType
    ALU = mybir.AluOpType

    tpad = sum(caps)

    nc = bacc.Bacc(None, target_bir_lowering=False)

    xt_d = nc.declare_dram_parameter("xt", [H, tpad], bf16, isOutput=False)
    wc_d = [
        nc.declare_dram_parameter(f"wc{e}", [128, 6 * HC[e]], bf16, isOutput=False)
        for e in range(3)
    ]
    wdb_d = [
        nc.declare_dram_parameter(f"wdb{e}", [128, MC[e] * H], bf16, isOutput=False)
        for e in range(3)
    ]
    bcp_d = nc.declare_dram_parameter("bcp", [128, 18], f32, isOutput=False)
    if apply_gb:
        gb_d = nc.declare_dram_parameter("gb", [2, H], f32, isOutput=False)
    out_d = nc.declare_dram_parameter("out", [tpad, H], bf16, isOutput=True)

    with tile.TileContext(nc) as tc:
        from contextlib import ExitStack

        with ExitStack() as ctx:
            wpool = ctx.enter_context(tc.tile_pool(name="weights", bufs=1))
            cpool = ctx.enter_context(tc.tile_pool(name="consts", bufs=1))
            xpool = ctx.enter_context(tc.tile_pool(name="xt", bufs=8))
            zpsum = ctx.enter_context(tc.tile_pool(name="zpsum", bufs=2, space="PSUM"))
            zpool = ctx.enter_context(tc.tile_pool(name="zsb", bufs=7))
            ypsum = ctx.enter_context(tc.tile_pool(name="ypsum", bufs=3, space="PSUM"))
            opool = ctx.enter_context(tc.tile_pool(name="osb", bufs=4))
            lnpool = ctx.enter_context(tc.tile_pool(name="ln", bufs=8))

            # ---- constants first (tiny; first z-copy needs bc) ----
            bc_sb = cpool.tile([128, 18], f32)
            nc.scalar.dma_start(out=bc_sb, in_=bcp_d[:, :])
            eps_t = cpool.tile([128, 1], f32)
            nc.vector.memset(eps_t, EPS)
            if apply_gb:
                gb_sb = cpool.tile([128, 2, H], f32)
                nc.scalar.dma_start(
                    out=gb_sb,
                    in_=gb_d.ap().partition_broadcast(128),
                )

            # ---- weight tiles; host-prepacked [128, ...] images, one DMA
            # each, interleaved into the first rounds on the sync queue ----
            wc_sb = [None] * 3
            wd_sb = [None] * 3
            for e in range(3):
                wc_sb[e] = wpool.tile(
                    [128, 6, HC[e]], bf16, tag=f"wc{e}", name=f"wc_sb{e}"
                )
                wd_sb[e] = wpool.tile(
                    [128, MC[e], H], bf16, tag=f"wd{e}", name=f"wd_sb{e}"
                )

            def _issue_wc(e, eng):
                eng.dma_start(
                    out=wc_sb[e],
                    in_=wc_d[e].ap().rearrange("p (c h) -> p c h", c=6),
                )

            def _issue_wd(e, eng):
                eng.dma_start(
                    out=wd_sb[e],
                    in_=wdb_d[e].ap().rearrange("p (k h) -> p k h", k=MC[e]),
                )

            # e2's tiny weights go first so the first real matmul starts as
            # soon as xt0 lands; the rest interleave into the next groups.
            order = _first_use_order(caps)
            _issue_wc(order[0], nc.sync)
            pending_weights = [lambda e2=order[0]: _issue_wd(e2, nc.sync)]
            for e in order[1:]:
                pending_weights.append(lambda e2=e: _issue_wc(e2, nc.sync))
                pending_weights.append(lambda e2=e: _issue_wd(e2, nc.sync))

            # PE warm-up: dummy matmuls during the initial weight DMA wait
            # keep the HAM activity window hot so real matmuls start at
            # full clock.
            warm = cpool.tile([128, 512], bf16, name="warm")
            nc.vector.memset(warm, 0.0)
            warm_ps = zpsum.tile([128, 512], f32, tag="pz", name="warm_ps")
            for _w in range(16):
                nc.tensor.matmul(
                    warm_ps,
                    lhsT=warm[:, 0:128],
                    rhs=warm,
                    start=(_w == 0),
                    stop=(_w == 15),
                )
            xt_r = xt_d.ap().rearrange("(c p) t -> p c t", p=128)

            # ---- program: flat weave of groups (e2 first, experts
            # interleaved); each group runs mm1 then mm2 back-to-back so
            # LayerNorm of one group hides under the next group's mm1.
            # mm2 subtiles run in PAIRS whose ragged tail K-chunks execute
            # concurrently as row-tiles at two partition offsets. ----
            order = _order(caps)
            subtile_no = 0

            def do_ln(py, o_t):
                nonlocal subtile_no
                stats = lnpool.tile([128, 2, 6], f32, tag="stats")
                for j in range(2):
                    nc.vector.bn_stats(
                        out=stats[:, j, :], in_=py[:, j * 384 : (j + 1) * 384]
                    )
                mv = lnpool.tile([128, 2], f32, tag="mv")
                nc.vector.bn_aggr(out=mv, in_=stats)
                rstd = lnpool.tile([128, 1], f32, tag="rstd")
                nc.scalar.activation(
                    out=rstd, in_=mv[:, 1:2], func=AF.Sqrt, bias=eps_t, scale=1.0
                )
                nc.vector.reciprocal(out=rstd, in_=rstd)
                subtile_no += 1
                if subtile_no % 3 != 0:
                    negmu = lnpool.tile([128, 1], f32, tag="negmu")
                    nc.vector.tensor_scalar(
                        out=negmu,
                        in0=mv[:, 0:1],
                        scalar1=rstd[:, 0:1],
                        scalar2=-1.0,
                        op0=ALU.mult,
                        op1=ALU.mult,
                    )
                    nc.scalar.activation(
                        out=o_t,
                        in_=py,
                        func=AF.Identity,
                        bias=negmu,
                        scale=rstd[:, 0:1],
                    )
                else:
                    nc.vector.tensor_scalar(
                        out=o_t,
                        in0=py,
                        scalar1=mv[:, 0:1],
                        scalar2=rstd[:, 0:1],
                        op0=ALU.subtract,
                        op1=ALU.mult,
                    )
                if apply_gb:
                    nc.gpsimd.tensor_tensor(
                        out=o_t, in0=o_t, in1=gb_sb[:, 0, :], op=ALU.mult
                    )
                    nc.vector.tensor_add(o_t, o_t, gb_sb[:, 1, :])

            for grp_no, (e, tok0, gsz) in enumerate(order):
                hc, fm, tm, tk, mc = HC[e], FM[e], TM[e], TK[e], MC[e]
                rs = 64 if e == 0 else (32 if e == 1 else 0)
                xt_t = xpool.tile([128, 6, gsz], bf16, tag="xt")
                nc.sync.dma_start(out=xt_t, in_=xt_r[:, :, tok0 : tok0 + gsz])
                for _ in range(2):
                    if pending_weights:
                        pending_weights.pop(0)()
                zt = zpool.tile([128, 6, gsz], bf16, tag="zt")
                # ones rows for the bd term: memset 32-aligned windows
                # covering partition tm (and rs+tm for the replica); the z
                # copies below overwrite the real z rows inside.
                w0 = (tm // 32) * 32
                nc.gpsimd.memset(zt[w0 : w0 + 32, fm, :], 1.0)
                if e != 2 and gsz > 128:
                    w1 = rs + (tm // 32) * 32
                    nc.gpsimd.memset(zt[w1 : w1 + 32, fm, :], 1.0)
                # mm1: full 128-col chunks plus the ragged tail chunk
                for m in range(mc):
                    hcm = min(128, hc - m * 128)
                    pz = zpsum.tile([128, gsz], f32, tag="pz")
                    for c in range(6):
                        nc.tensor.matmul(
                            pz[0:hcm, :],
                            lhsT=wc_sb[e][:, c, m * 128 : m * 128 + hcm],
                            rhs=xt_t[:, c, :],
                            start=(c == 0),
                            stop=(c == 5),
                        )
                    nc.scalar.activation(
                        out=zt[0:hcm, m, :],
                        in_=pz[0:hcm, :],
                        func=AF.Identity,
                        bias=bc_sb[0:hcm, e * 6 + m : e * 6 + m + 1],
                        scale=1.0,
                    )
                if e != 2 and gsz > 128:
                    # replicate the (biased) z tail rows at the second
                    # row-strip offset (partition-shifted ACT copy) for the
                    # paired mm2 tails
                    nc.scalar.activation(
                        out=zt[rs : rs + tm, fm, :],
                        in_=zt[0:tm, fm, :],
                        func=AF.Identity,
                        scale=1.0,
                    )
                # mm2 with LayerNorm, subtiles in pairs
                nsub = gsz // 128
                o_g = opool.tile([128, nsub, H], bf16, tag="o")
                for s0 in range(0, nsub, 2):
                    sbi = s0 + 1 if s0 + 1 < nsub else None
                    pya = ypsum.tile([128, H], f32, tag="py", name="pya")
                    if sbi is not None:
                        pyb = ypsum.tile([128, H], f32, tag="py", name="pyb")
                    else:
                        pyb = None
                    subs = [(s0, pya, 0)] + (
                        [(sbi, pyb, rs)] if sbi is not None else []
                    )
                    for s, py, _r in subs:
                        for k in range(fm):
                            for n0, nn in ((0, 512), (512, 256)):
                                nc.tensor.matmul(
                                    py[:, n0 : n0 + nn],
                                    lhsT=zt[:, k, s * 128 : (s + 1) * 128],
                                    rhs=wd_sb[e][:, k, n0 : n0 + nn],
                                    start=(k == 0),
                                    stop=False,
                                )
                    # paired ragged tail K-chunks: concurrent row-tiles at
                    # partition offsets 0 and rs, separate PSUM banks
                    for n0, nn in ((0, 512), (512, 256)):
                        for s, py, r in subs:
                            nc.tensor.matmul(
                                py[:, n0 : n0 + nn],
                                lhsT=zt[r : r + tk, fm, s * 128 : (s + 1) * 128],
                                rhs=wd_sb[e][r : r + tk, fm, n0 : n0 + nn],
                                start=(fm == 0),
                                stop=True,
                                tile_position=(r, 0),
                            )
                    for s, py, _r in subs:
                        do_ln(py, o_g[:, s, :])
                nc.scalar.dma_start(
                    out=out_d[tok0 : tok0 + gsz, :].rearrange(
                        "(s p) h -> p s h", p=128
                    ),
                    in_=o_g,
                )
    nc.finalize()
    return nc


def _get_nc(apply_gb: bool, caps):
    key = (apply_gb, caps)
    if key not in _NC_CACHE:
        _NC_CACHE[key] = _build(apply_gb, caps=caps)
    return _NC_CACHE[key]


def _pack_weights(inputs):
    base = {}
    bcp = np.zeros((128, 18), np.float32)
    for e in range(3):
        hc, fm, tm, mc = HC[e], FM[e], TM[e], MC[e]
        offsets = [0, 64] if e == 0 else ([0, 32, 64, 96] if e == 1 else [0])
        wc = np.asarray(inputs[f"Wc{e}"], dtype=np.float32)  # [H, hc]
        bc = np.asarray(inputs[f"bc{e}"], dtype=np.float32)
        wd = np.asarray(inputs[f"Wd{e}"], dtype=np.float32)  # [hc, H]
        bd = np.asarray(inputs[f"bd{e}"], dtype=np.float32)
        # wc image [128, 6, hc]: [p, c, j] = Wc[c*128+p, j]
        wci = wc.reshape(6, 128, hc).transpose(1, 0, 2)
        base[f"wc{e}"] = np.ascontiguousarray(
            wci.reshape(128, 6 * hc).astype(BF16)
        )
        # wd image [128, mc, H]: full chunks, then tail rows (+bd ones-row)
        # replicated at every row-strip offset
        wdi = np.zeros((128, mc, H), np.float32)
        for k in range(fm):
            wdi[:, k, :] = wd[k * 128 : (k + 1) * 128]
        for off in offsets:
            wdi[off : off + tm, fm, :] = wd[fm * 128 :]
            wdi[off + tm, fm, :] = bd
        base[f"wdb{e}"] = np.ascontiguousarray(
            wdi.reshape(128, mc * H).astype(BF16)
        )
        # packed bc: column e*6+m; tail column replicated at offsets
        for m in range(fm):
            bcp[:, e * 6 + m] = bc[m * 128 : (m + 1) * 128]
        for off in offsets:
            bcp[off : off + tm, e * 6 + fm] = bc[fm * 128 :]
    base["bcp"] = bcp
    return base


def kernel(**inputs):
    global LAST_RESULT
    from concourse.bass_utils import run_bass_kernel_spmd

    hs = np.ascontiguousarray(np.asarray(inputs["hidden_states"], dtype=np.float32))
    sc = np.asarray(inputs["importance_scores"], dtype=np.float32)
    gamma = np.asarray(inputs["gamma"], dtype=np.float32)
    beta = np.asarray(inputs["beta"], dtype=np.float32)

    # routing (must match f32 comparison semantics of the reference)
    m0 = sc > np.float32(0.8)
    m1 = (sc > np.float32(0.4)) & ~m0
    bucket = np.where(m0, 0, np.where(m1, 1, 2)).astype(np.int64)
    idx = [np.flatnonzero(bucket == e) for e in range(3)]
    splits = [np.array_split(idx[e], N_CORES) for e in range(3)]

    # tight per-core caps: max per-core count rounded up to 128
    caps = tuple(
        int(-(-max(len(p) for p in splits[e]) // 128) * 128) for e in range(3)
    )
    tpad = sum(caps)
    offs = (0, caps[0], caps[0] + caps[1])

    gidx = np.zeros((N_CORES, tpad), np.int64)
    valid = np.zeros((N_CORES, tpad), bool)
    for c in range(N_CORES):
        for e in range(3):
            p = splits[e][c]
            o = offs[e]
            gidx[c, o : o + len(p)] = p
            valid[c, o : o + len(p)] = True

    apply_gb = not (np.all(gamma == 1.0) and np.all(beta == 0.0))
    nc = _get_nc(apply_gb, caps)

    base = _pack_weights(inputs)
    if apply_gb:
        base["gb"] = np.ascontiguousarray(np.stack([gamma, beta], axis=0))

    in_maps = []
    for c in range(N_CORES):
        xc = hs[gidx[c]]  # [TPAD, H]
        m = dict(base)
        m["xt"] = np.ascontiguousarray(xc.T.astype(BF16))
        in_maps.append(m)

    # The device occasionally returns corrupted (non-finite) results or
    # raises an unrecoverable-state error; inputs are finite and LayerNorm
    # output is always finite, so retry in both cases.
    for attempt in range(4):
        try:
            res = run_bass_kernel_spmd(
                nc, in_maps, core_ids=list(range(N_CORES)), trace=TRACE
            )
        except Exception:
            if attempt == 3:
                raise
            import time as _time

            _time.sleep(2.0)
            continue
        LAST_RESULT = res
        out = np.empty((S, H), np.float32)
        for c in range(N_CORES):
            v = valid[c]
            out[gidx[c][v]] = res.results[c]["out"][v]
        if np.isfinite(out).all():
            break
    return out

